# revision 3
# baseline (speedup 1.0000x reference)
"""KNN column-imputation kernel (nn_ColProcessor) for 8 Trainium2 cores.

Strategy: shard the 4096 query rows across 8 cores (512 rows each, data
parallel). The query-row distances are pre-encoded on the host as bf16
"codes" of -d (truncation of the f32 bits + sign flip), a monotone map, so
the device streams 2 bytes/element instead of 4 (halving HBM traffic) and
the vector engine runs its fold in the 2-byte 2x performance mode.

Each core processes its [512, 16384] code slice in [128, 16384] tiles:
  - DMA in: 4 column-chunks of 1 MiB per tile, alternating between the two
    HWDGE rings (SP via nc.sync, Activation via nc.scalar) — a single ring
    saturates at ~230 GB/s, both together approach the ~435 GB/s SBUF
    fabric ceiling, and >=1 MiB per DMA keeps descriptor overhead small.
  - DVE fold: 5 tensor_tensor max halvings 16384 -> 512 "block maxima"
    (block b = comb {b + 512k, k=0..31}). bf16 runs in the 2x_1p DVE mode
    (2 elem/cycle) and each halving costs only its output width: ~7936
    cycles/tile vs 16384+16384 for a full-width max+max_index.
  - DMA out: the 512 block codes per row, via the gpsimd SWDGE ring so the
    HWDGE rings never stall on a compute semaphore (HWDGE DMAs execute
    FIFO per issuing engine - an out-DMA waiting on the fold would block
    the next tile's input DMAs).

Host side: top-12 blocks per row by code (argpartition), gather the 12x32
candidate f32 distances, re-rank exactly by (value, global index) -
identical to jax.lax.top_k's tie-break - take the first 5 donors, and
certify: every chosen donor's code must be strictly better than the 12th
block code (nothing in an unselected block can beat it) and at least 5
donors must be present. Rows failing the certificate (~30/4096) are
replayed exactly on the host. Certified rows match the reference
bit-for-bit, so the result is exact, not approximate.
"""

import sys

sys.path.insert(0, "/opt/trn_rl_repo")

import numpy as np
import ml_dtypes

import concourse.bacc as bacc
import concourse.mybir as mybir
from concourse.tile import TileContext

N_Q, N_FIT, N_FEAT = 4096, 16384, 32
COL, K = 3, 5
BIG = 1.0e30
NAN_FILL = 1.0e10
N_CORES = 8
ROWS = N_Q // N_CORES  # 512 query rows per core
P = 128
N_TILES = ROWS // P  # 4
NCHUNK = 4  # column-chunks for DMA (1 MiB each), alternating SP/Act HWDGE rings
W = 512  # block count after the fold (blocks are combs {b + 512k})
TEETH = N_FIT // W  # 32
NSEL = 12  # blocks gathered per row on the host

_EXEC_CACHE = {}


def _build(loop_n=None):
    """Build the per-core NEFF. loop_n wraps the body in an on-device For_i
    loop (used only for timing: the NEFF size is loop-bound independent, so
    wall-clock slopes between loop counts isolate pure HW execution time)."""
    import contextlib

    nc = bacc.Bacc("TRN2", target_bir_lowering=False)
    c_in = nc.dram_tensor("c", [ROWS, N_FIT], mybir.dt.bfloat16, kind="ExternalInput")
    if loop_n:
        # timing-only builds take a per-call-unique salt so the axon relay's
        # identical-execution cache can't elide repeated timed runs
        salt_in = nc.dram_tensor("salt", [1, 8], mybir.dt.float32, kind="ExternalInput")
    v_out = nc.dram_tensor("vals", [ROWS, W], mybir.dt.bfloat16, kind="ExternalOutput")
    CW = N_FIT // NCHUNK

    with TileContext(nc) as tc:
        with (
            tc.tile_pool(name="work", bufs=2) as work,
            tc.tile_pool(name="fold", bufs=2) as fold,
            tc.tile_pool(name="small", bufs=4) as small,
        ):
            if loop_n:
                salt_t = small.tile([1, 8], mybir.dt.float32)
                nc.sync.dma_start(out=salt_t, in_=salt_in[:, :])
            loop = tc.For_i(0, loop_n, 1) if loop_n else contextlib.nullcontext()
            with loop:
                for t in range(N_TILES):
                    rs = slice(t * P, (t + 1) * P)
                    dt = work.tile([P, N_FIT], mybir.dt.bfloat16)
                    for ch in range(NCHUNK):
                        cs = slice(ch * CW, (ch + 1) * CW)
                        eng = (nc.sync, nc.scalar)[ch % 2]
                        eng.dma_start(out=dt[:, cs], in_=c_in[rs, cs])
                    f1 = fold.tile([P, 8192], mybir.dt.bfloat16)
                    f2 = fold.tile([P, 4096], mybir.dt.bfloat16)
                    f3 = fold.tile([P, 2048], mybir.dt.bfloat16)
                    f4 = fold.tile([P, 1024], mybir.dt.bfloat16)
                    f5 = fold.tile([P, W], mybir.dt.bfloat16)
                    # fold1 pairs {i, i+4096} within each half so each fold1
                    # instruction depends on only half the chunks; fold2
                    # merges the halves, and fold3..5 keep halving, so block
                    # b ends up as the comb {b + 512k, k=0..31}.
                    nc.vector.tensor_max(
                        out=f1[:, 0:4096], in0=dt[:, 0:4096], in1=dt[:, 4096:8192]
                    )
                    nc.vector.tensor_max(
                        out=f1[:, 4096:8192], in0=dt[:, 8192:12288], in1=dt[:, 12288:16384]
                    )
                    nc.vector.tensor_max(out=f2, in0=f1[:, 0:4096], in1=f1[:, 4096:8192])
                    nc.vector.tensor_max(out=f3, in0=f2[:, 0:2048], in1=f2[:, 2048:4096])
                    nc.vector.tensor_max(out=f4, in0=f3[:, 0:1024], in1=f3[:, 1024:2048])
                    nc.vector.tensor_max(out=f5, in0=f4[:, 0:512], in1=f4[:, 512:1024])
                    nc.gpsimd.dma_start(out=v_out[rs, :], in_=f5)
    nc.finalize()
    return nc


def _get_exec(nc):
    """Cached jitted 8-core executor for a finalized Bass module.

    Mirrors bass2jax.run_bass_via_pjrt's multi-core path but memoizes the
    jitted function so repeated calls don't re-trace/re-compile, and accepts
    already-device-resident concat inputs.
    """
    key = id(nc)
    if key in _EXEC_CACHE:
        return _EXEC_CACHE[key]

    import jax
    from jax.sharding import Mesh, PartitionSpec
    from jax.experimental.shard_map import shard_map
    from concourse import bass2jax
    from concourse import mybir as _mybir

    bass2jax.install_neuronx_cc_hook()

    partition_name = nc.partition_id_tensor.name if nc.partition_id_tensor else None
    in_names, out_names, out_avals, zero_outs = [], [], [], []
    for alloc in nc.m.functions[0].allocations:
        if not isinstance(alloc, _mybir.MemoryLocationSet):
            continue
        name = alloc.memorylocations[0].name
        if alloc.kind == "ExternalInput":
            if name != partition_name:
                in_names.append(name)
        elif alloc.kind == "ExternalOutput":
            out_names.append(name)
            shape = tuple(alloc.tensor_shape)
            dtype = _mybir.dt.np(alloc.dtype)
            out_avals.append(jax.core.ShapedArray(shape, dtype))
            zero_outs.append(np.zeros(shape, dtype))
    n_params = len(in_names)
    n_outs = len(out_avals)
    all_in_names = list(in_names) + list(out_names)
    if partition_name is not None:
        all_in_names.append(partition_name)
    donate = tuple(range(n_params, n_params + n_outs))

    def _body(*args):
        operands = list(args)
        if partition_name is not None:
            operands.append(bass2jax.partition_id_tensor())
        outs = bass2jax._bass_exec_p.bind(
            *operands,
            out_avals=tuple(out_avals),
            in_names=tuple(all_in_names),
            out_names=tuple(out_names),
            lowering_input_output_aliases=(),
            sim_require_finite=True,
            sim_require_nnan=True,
            nc=nc,
        )
        return tuple(outs)

    devices = jax.devices()[:N_CORES]
    mesh = Mesh(np.asarray(devices), ("core",))
    in_specs = (PartitionSpec("core"),) * (n_params + n_outs)
    out_specs = (PartitionSpec("core"),) * n_outs
    jitted = jax.jit(
        shard_map(
            _body, mesh=mesh, in_specs=in_specs, out_specs=out_specs, check_rep=False
        ),
        donate_argnums=donate,
        keep_unused=True,
    )

    def run(concat_inputs):
        """concat_inputs: dict name -> (N_CORES*per_core_rows, ...) array."""
        args = [concat_inputs[n] for n in in_names]
        zeros = [
            np.zeros((N_CORES * z.shape[0], *z.shape[1:]), z.dtype) for z in zero_outs
        ]
        outs = jitted(*args, *zeros)
        return {n: outs[i] for i, n in enumerate(out_names)}

    _EXEC_CACHE[key] = run
    return run


_NC = None


def make_codes(d):
    """bf16 codes of -d: truncate the f32 bits to bf16 and set the sign bit.
    Monotone non-increasing in d, so max over codes = min over distances."""
    u = (np.ascontiguousarray(d).view(np.uint32) >> np.uint32(16)).astype(np.uint16)
    u |= np.uint16(0x8000)
    return u.view(ml_dtypes.bfloat16)


def _device_block_codes(codes):
    """codes: [N_Q, N_FIT] bf16 -> block codes [N_Q, W] bf16 (max over combs)."""
    global _NC
    if _NC is None:
        _NC = _build()
    run = _get_exec(_NC)
    out = run({"c": np.ascontiguousarray(codes)})
    return np.asarray(out["vals"])


def _exact_rows(d_rows, donor_ok, mask_fit_col, fitcol):
    """Exact numpy replay of the reference for a few rows: returns val[n]."""
    dm = np.where(
        donor_ok[None, :],
        np.where(np.isnan(d_rows), np.float32(NAN_FILL), d_rows),
        np.float32(BIG),
    )
    all_nan = np.all(np.isnan(d_rows) | ~donor_ok[None, :], axis=1)
    order = np.argsort(dm, axis=1, kind="stable")[:, :K]
    w = 1.0 - mask_fit_col[order].astype(np.float32)
    donors = fitcol[order]
    wsum = w.sum(axis=1)
    div = np.where(wsum == 0, np.float32(1.0), wsum)
    knn_val = (donors * w).sum(axis=1) / div
    obs = ~mask_fit_col
    msum = obs.sum(dtype=np.float32)
    col_sum = np.where(obs, fitcol, 0.0).sum(dtype=np.float32)
    col_mean = col_sum / (msum if msum > 0 else np.float32(1.0))
    return np.where(all_nan, col_mean, knn_val).astype(np.float32)


def kernel(
    X,
    dist_chunk,
    non_missing_fix_X,
    mask_fit_X,
    dist_idx_map,
    mask,
    row_missing_idx,
    _fit_X,
):
    X = np.asarray(X, dtype=np.float32)
    dist_chunk = np.asarray(dist_chunk, dtype=np.float32)
    non_missing_fix_X = np.asarray(non_missing_fix_X, dtype=bool)
    mask_fit_X = np.asarray(mask_fit_X, dtype=bool)
    mask = np.asarray(mask, dtype=bool)
    _fit_X = np.asarray(_fit_X, dtype=np.float32)
    rmi = np.asarray(row_missing_idx, dtype=np.int64)
    dmap = np.asarray(dist_idx_map, dtype=np.int64)

    gather_rows = dmap[rmi]
    if gather_rows.shape[0] == N_Q and np.array_equal(
        gather_rows, np.arange(N_Q, dtype=np.int64)
    ):
        d = np.ascontiguousarray(dist_chunk)
    else:
        d = np.ascontiguousarray(dist_chunk[gather_rows])
    assert d.shape == (N_Q, N_FIT)

    codes = make_codes(d)
    bc = _device_block_codes(codes)

    donor_ok = non_missing_fix_X[:, COL]
    fitcol = _fit_X[:, COL]
    mask_fit_col = mask_fit_X[:, COL]

    # host-side block selection: NSEL smallest block codes per row. The codes
    # are negative bf16, so float-greater (= smaller distance) == uint16-less.
    bu = np.ascontiguousarray(bc).view(np.uint16)
    part = np.argpartition(bu, NSEL - 1, axis=1)[:, :NSEL]
    thr = np.take_along_axis(bu, part, axis=1).max(axis=1)  # NSEL-th block code

    # gather the candidate blocks (32 comb teeth each) per row
    gidx = (
        part[:, :, None].astype(np.int64) + W * np.arange(TEETH, dtype=np.int64)[None, None, :]
    ).reshape(N_Q, NSEL * TEETH)
    dv = np.take_along_axis(d, gidx, axis=1)  # exact f32 distances
    cu = np.take_along_axis(codes.view(np.uint16), gidx, axis=1)  # candidate codes

    # order candidates by global index, then stable-sort by (donor-masked)
    # value: equal values resolve to the lowest index, same as jax.lax.top_k
    perm = np.argsort(gidx, axis=1, kind="stable")
    gidx_s = np.take_along_axis(gidx, perm, axis=1)
    dv_s = np.take_along_axis(dv, perm, axis=1)
    cu_s = np.take_along_axis(cu, perm, axis=1)
    donor_s = donor_ok[gidx_s]
    dv_inf = np.where(donor_s, dv_s, np.float32(np.inf))
    sel = np.argsort(dv_inf, axis=1, kind="stable")[:, :K]
    idx5 = np.take_along_axis(gidx_s, sel, axis=1)
    c5u = np.take_along_axis(cu_s, sel, axis=1)
    have5 = np.take_along_axis(dv_inf, sel, axis=1)[:, K - 1] < np.inf

    # certificate: every chosen donor's code strictly beats the NSEL-th block
    # code; otherwise an element of an unselected block could displace it.
    cert = (c5u < thr[:, None]).all(axis=1) & have5
    bad_rows = ~cert

    w = 1.0 - mask_fit_col[idx5].astype(np.float32)
    donors = fitcol[idx5]
    wsum = w.sum(axis=1)
    div = np.where(wsum == 0, np.float32(1.0), wsum)
    val = (donors * w).sum(axis=1) / div

    if bad_rows.any():
        bad = np.flatnonzero(bad_rows)
        val[bad] = _exact_rows(d[bad], donor_ok, mask_fit_col, fitcol)

    col_mask = mask[rmi, COL]
    new_col = np.where(col_mask, val, X[rmi, COL]).astype(np.float32)
    out = X.copy()
    out[rmi, COL] = new_col
    return out


# revision 4
# speedup vs baseline: 1.1335x; 1.1335x over previous
"""KNN column-imputation kernel (nn_ColProcessor) for 8 Trainium2 cores.

Strategy: shard the 4096 query rows across 8 cores (512 rows each, data
parallel). The query-row distances are pre-encoded on the host as bf16
"codes" of -d (truncation of the f32 bits + sign flip), a monotone map, so
the device streams 2 bytes/element instead of 4 (halving HBM traffic) and
the vector engine runs its fold in the 2-byte 2x performance mode.

Each core processes its [512, 16384] code slice in [128, 16384] tiles:
  - DMA in: 4 column-chunks of 1 MiB per tile, alternating between the two
    HWDGE rings (SP via nc.sync, Activation via nc.scalar) — a single ring
    saturates at ~230 GB/s, both together approach the ~435 GB/s SBUF
    fabric ceiling, and >=1 MiB per DMA keeps descriptor overhead small.
  - DVE fold: 5 tensor_tensor max halvings 16384 -> 512 "block maxima"
    (block b = comb {b + 512k, k=0..31}). bf16 runs in the 2x_1p DVE mode
    (2 elem/cycle) and each halving costs only its output width: ~7936
    cycles/tile vs 16384+16384 for a full-width max+max_index.
  - DMA out: the 512 block codes per row, via the gpsimd SWDGE ring so the
    HWDGE rings never stall on a compute semaphore (HWDGE DMAs execute
    FIFO per issuing engine - an out-DMA waiting on the fold would block
    the next tile's input DMAs).

Host side: top-12 blocks per row by code (argpartition), gather the 12x32
candidate f32 distances, re-rank exactly by (value, global index) -
identical to jax.lax.top_k's tie-break - take the first 5 donors, and
certify: every chosen donor's code must be strictly better than the 12th
block code (nothing in an unselected block can beat it) and at least 5
donors must be present. Rows failing the certificate (~30/4096) are
replayed exactly on the host. Certified rows match the reference
bit-for-bit, so the result is exact, not approximate.
"""

import sys

sys.path.insert(0, "/opt/trn_rl_repo")

import numpy as np
import ml_dtypes

import concourse.bacc as bacc
import concourse.mybir as mybir
from concourse.tile import TileContext

N_Q, N_FIT, N_FEAT = 4096, 16384, 32
COL, K = 3, 5
BIG = 1.0e30
NAN_FILL = 1.0e10
N_CORES = 8
ROWS = N_Q // N_CORES  # 512 query rows per core
P = 128
N_TILES = ROWS // P  # 4
NCHUNK = 4  # column-chunks for DMA (1 MiB each), alternating SP/Act HWDGE rings
W = 512  # block count after the fold (blocks are combs {b + 512k})
TEETH = N_FIT // W  # 32
NSEL = 12  # blocks gathered per row on the host

_EXEC_CACHE = {}


def _build(loop_n=None):
    """Build the per-core NEFF. loop_n wraps the body in an on-device For_i
    loop (used only for timing: the NEFF size is loop-bound independent, so
    wall-clock slopes between loop counts isolate pure HW execution time)."""
    import contextlib

    nc = bacc.Bacc("TRN2", target_bir_lowering=False)
    c_in = nc.dram_tensor("c", [ROWS, N_FIT], mybir.dt.bfloat16, kind="ExternalInput")
    if loop_n:
        # timing-only builds take a per-call-unique salt so the axon relay's
        # identical-execution cache can't elide repeated timed runs
        salt_in = nc.dram_tensor("salt", [1, 8], mybir.dt.float32, kind="ExternalInput")
    v_out = nc.dram_tensor("vals", [ROWS, W], mybir.dt.bfloat16, kind="ExternalOutput")
    CW = N_FIT // NCHUNK

    with TileContext(nc) as tc:
        with (
            tc.tile_pool(name="work", bufs=2) as work,
            tc.tile_pool(name="fold", bufs=2) as fold,
            tc.tile_pool(name="small", bufs=4) as small,
        ):
            if loop_n:
                salt_t = small.tile([1, 8], mybir.dt.float32)
                nc.sync.dma_start(out=salt_t, in_=salt_in[:, :])
            loop = tc.For_i(0, loop_n, 1) if loop_n else contextlib.nullcontext()
            with loop:
                for t in range(N_TILES):
                    rs = slice(t * P, (t + 1) * P)
                    dt = work.tile([P, N_FIT], mybir.dt.bfloat16)
                    for ch in range(NCHUNK):
                        cs = slice(ch * CW, (ch + 1) * CW)
                        eng = (nc.sync, nc.scalar)[ch % 2]
                        eng.dma_start(out=dt[:, cs], in_=c_in[rs, cs])
                    f1 = fold.tile([P, 8192], mybir.dt.bfloat16)
                    f2 = fold.tile([P, 4096], mybir.dt.bfloat16)
                    f3 = fold.tile([P, 2048], mybir.dt.bfloat16)
                    f4 = fold.tile([P, 1024], mybir.dt.bfloat16)
                    f5 = fold.tile([P, W], mybir.dt.bfloat16)
                    # fold1 pairs {i, i+4096} within each half so each fold1
                    # instruction depends on only half the chunks; fold2
                    # merges the halves, and fold3..5 keep halving, so block
                    # b ends up as the comb {b + 512k, k=0..31}.
                    nc.vector.tensor_max(
                        out=f1[:, 0:4096], in0=dt[:, 0:4096], in1=dt[:, 4096:8192]
                    )
                    nc.vector.tensor_max(
                        out=f1[:, 4096:8192], in0=dt[:, 8192:12288], in1=dt[:, 12288:16384]
                    )
                    nc.vector.tensor_max(out=f2, in0=f1[:, 0:4096], in1=f1[:, 4096:8192])
                    nc.vector.tensor_max(out=f3, in0=f2[:, 0:2048], in1=f2[:, 2048:4096])
                    nc.vector.tensor_max(out=f4, in0=f3[:, 0:1024], in1=f3[:, 1024:2048])
                    nc.vector.tensor_max(out=f5, in0=f4[:, 0:512], in1=f4[:, 512:1024])
                    nc.gpsimd.dma_start(out=v_out[rs, :], in_=f5)
    nc.finalize()
    return nc


def _get_exec(nc):
    """Cached jitted 8-core executor for a finalized Bass module.

    Mirrors bass2jax.run_bass_via_pjrt's multi-core path but memoizes the
    jitted function so repeated calls don't re-trace/re-compile, and accepts
    already-device-resident concat inputs.
    """
    key = id(nc)
    if key in _EXEC_CACHE:
        return _EXEC_CACHE[key]

    import jax
    from jax.sharding import Mesh, PartitionSpec
    from jax.experimental.shard_map import shard_map
    from concourse import bass2jax
    from concourse import mybir as _mybir

    bass2jax.install_neuronx_cc_hook()

    partition_name = nc.partition_id_tensor.name if nc.partition_id_tensor else None
    in_names, out_names, out_avals, zero_outs = [], [], [], []
    for alloc in nc.m.functions[0].allocations:
        if not isinstance(alloc, _mybir.MemoryLocationSet):
            continue
        name = alloc.memorylocations[0].name
        if alloc.kind == "ExternalInput":
            if name != partition_name:
                in_names.append(name)
        elif alloc.kind == "ExternalOutput":
            out_names.append(name)
            shape = tuple(alloc.tensor_shape)
            dtype = _mybir.dt.np(alloc.dtype)
            out_avals.append(jax.core.ShapedArray(shape, dtype))
            zero_outs.append(np.zeros(shape, dtype))
    n_params = len(in_names)
    n_outs = len(out_avals)
    all_in_names = list(in_names) + list(out_names)
    if partition_name is not None:
        all_in_names.append(partition_name)
    donate = tuple(range(n_params, n_params + n_outs))

    def _body(*args):
        operands = list(args)
        if partition_name is not None:
            operands.append(bass2jax.partition_id_tensor())
        outs = bass2jax._bass_exec_p.bind(
            *operands,
            out_avals=tuple(out_avals),
            in_names=tuple(all_in_names),
            out_names=tuple(out_names),
            lowering_input_output_aliases=(),
            sim_require_finite=True,
            sim_require_nnan=True,
            nc=nc,
        )
        return tuple(outs)

    devices = jax.devices()[:N_CORES]
    mesh = Mesh(np.asarray(devices), ("core",))
    in_specs = (PartitionSpec("core"),) * (n_params + n_outs)
    out_specs = (PartitionSpec("core"),) * n_outs
    jitted = jax.jit(
        shard_map(
            _body, mesh=mesh, in_specs=in_specs, out_specs=out_specs, check_rep=False
        ),
        donate_argnums=donate,
        keep_unused=True,
    )

    def run(concat_inputs):
        """concat_inputs: dict name -> (N_CORES*per_core_rows, ...) array."""
        args = [concat_inputs[n] for n in in_names]
        zeros = [
            np.zeros((N_CORES * z.shape[0], *z.shape[1:]), z.dtype) for z in zero_outs
        ]
        outs = jitted(*args, *zeros)
        return {n: outs[i] for i, n in enumerate(out_names)}

    _EXEC_CACHE[key] = run
    return run


_NC = None


def make_codes(d):
    """bf16 codes of -d: truncate the f32 bits to bf16 and set the sign bit.
    Monotone non-increasing in d, so max over codes = min over distances.
    The odd uint16 halfwords of a little-endian f32 array are its high bits."""
    u = np.ascontiguousarray(d).view(np.uint16)[:, 1::2] | np.uint16(0x8000)
    return u.view(ml_dtypes.bfloat16)


def _device_block_codes(codes):
    """codes: [N_Q, N_FIT] bf16 -> block codes [N_Q, W] bf16 (max over combs)."""
    global _NC
    if _NC is None:
        _NC = _build()
    run = _get_exec(_NC)
    out = run({"c": np.ascontiguousarray(codes)})
    return np.asarray(out["vals"])


def _exact_rows(d_rows, donor_ok, mask_fit_col, fitcol):
    """Exact numpy replay of the reference for a few rows: returns val[n]."""
    dm = np.where(
        donor_ok[None, :],
        np.where(np.isnan(d_rows), np.float32(NAN_FILL), d_rows),
        np.float32(BIG),
    )
    all_nan = np.all(np.isnan(d_rows) | ~donor_ok[None, :], axis=1)
    order = np.argsort(dm, axis=1, kind="stable")[:, :K]
    w = 1.0 - mask_fit_col[order].astype(np.float32)
    donors = fitcol[order]
    wsum = w.sum(axis=1)
    div = np.where(wsum == 0, np.float32(1.0), wsum)
    knn_val = (donors * w).sum(axis=1) / div
    obs = ~mask_fit_col
    msum = obs.sum(dtype=np.float32)
    col_sum = np.where(obs, fitcol, 0.0).sum(dtype=np.float32)
    col_mean = col_sum / (msum if msum > 0 else np.float32(1.0))
    return np.where(all_nan, col_mean, knn_val).astype(np.float32)


def kernel(
    X,
    dist_chunk,
    non_missing_fix_X,
    mask_fit_X,
    dist_idx_map,
    mask,
    row_missing_idx,
    _fit_X,
):
    X = np.asarray(X, dtype=np.float32)
    dist_chunk = np.asarray(dist_chunk, dtype=np.float32)
    non_missing_fix_X = np.asarray(non_missing_fix_X, dtype=bool)
    mask_fit_X = np.asarray(mask_fit_X, dtype=bool)
    mask = np.asarray(mask, dtype=bool)
    _fit_X = np.asarray(_fit_X, dtype=np.float32)
    rmi = np.asarray(row_missing_idx, dtype=np.int64)
    dmap = np.asarray(dist_idx_map, dtype=np.int64)

    gather_rows = dmap[rmi]
    if gather_rows.shape[0] == N_Q and np.array_equal(
        gather_rows, np.arange(N_Q, dtype=np.int64)
    ):
        d = np.ascontiguousarray(dist_chunk)
    else:
        d = np.ascontiguousarray(dist_chunk[gather_rows])
    assert d.shape == (N_Q, N_FIT)

    codes = make_codes(d)
    bc = _device_block_codes(codes)

    donor_ok = non_missing_fix_X[:, COL]
    fitcol = _fit_X[:, COL]
    mask_fit_col = mask_fit_X[:, COL]

    # host-side block selection: NSEL smallest block codes per row. The codes
    # are negative bf16, so float-greater (= smaller distance) == uint16-less.
    bu = np.ascontiguousarray(bc).view(np.uint16)
    part = np.argpartition(bu, NSEL - 1, axis=1)[:, :NSEL]
    thr = np.take_along_axis(bu, part, axis=1).max(axis=1)  # NSEL-th block code

    # gather the candidate blocks (32 comb teeth each) per row
    gidx = (
        part[:, :, None].astype(np.int64) + W * np.arange(TEETH, dtype=np.int64)[None, None, :]
    ).reshape(N_Q, NSEL * TEETH)
    dv = np.take_along_axis(d, gidx, axis=1)  # exact f32 distances
    cu = np.take_along_axis(codes.view(np.uint16), gidx, axis=1)  # candidate codes

    # order candidates by global index, then stable-sort by (donor-masked)
    # value: equal values resolve to the lowest index, same as jax.lax.top_k
    perm = np.argsort(gidx, axis=1, kind="stable")
    gidx_s = np.take_along_axis(gidx, perm, axis=1)
    dv_s = np.take_along_axis(dv, perm, axis=1)
    cu_s = np.take_along_axis(cu, perm, axis=1)
    donor_s = donor_ok[gidx_s]
    dv_inf = np.where(donor_s, dv_s, np.float32(np.inf))
    sel = np.argsort(dv_inf, axis=1, kind="stable")[:, :K]
    idx5 = np.take_along_axis(gidx_s, sel, axis=1)
    c5u = np.take_along_axis(cu_s, sel, axis=1)
    have5 = np.take_along_axis(dv_inf, sel, axis=1)[:, K - 1] < np.inf

    # certificate: every chosen donor's code strictly beats the NSEL-th block
    # code; otherwise an element of an unselected block could displace it.
    cert = (c5u < thr[:, None]).all(axis=1) & have5
    bad_rows = ~cert

    w = 1.0 - mask_fit_col[idx5].astype(np.float32)
    donors = fitcol[idx5]
    wsum = w.sum(axis=1)
    div = np.where(wsum == 0, np.float32(1.0), wsum)
    val = (donors * w).sum(axis=1) / div

    if bad_rows.any():
        bad = np.flatnonzero(bad_rows)
        val[bad] = _exact_rows(d[bad], donor_ok, mask_fit_col, fitcol)

    col_mask = mask[rmi, COL]
    new_col = np.where(col_mask, val, X[rmi, COL]).astype(np.float32)
    out = X.copy()
    out[rmi, COL] = new_col
    return out
